# revision 1
# baseline (speedup 1.0000x reference)
"""Trainium2 Bass kernel: out = softmax(gelu_tanh(x @ W^T), axis=-1) + bias.

Full shapes: x [8192, 4096] f32, weight [4096, 4096] f32, bias [4096] f32.
Sharding: data-parallel over rows of x across 8 NeuronCores (1024 rows/core);
weight and bias replicated. Matmul runs in bf16 on the PE array with fp32
PSUM accumulation; gelu is computed with the exact tanh-approx constants of
the reference via DVE + ACT(Tanh), and softmax needs no max-subtraction
because gelu output is bounded in [-0.17, ~3.5] so exp cannot overflow.

Per-core loop structure (MC=1024 rows):
  split rows into G=2 groups of 512; for each group, stream weight n-tiles
  (512 cols) with the x-group resident in SBUF; accumulate 32 k-matmuls into
  PSUM per (m-tile, n-tile); fuse exp(gelu(v)) into the PSUM->SBUF epilogue
  with per-row sums accumulated by the ACT engine; normalize + bias-add with
  one fused DVE op per tile, then DMA out.

Measured on trn2 (8 cores): ~495 us HW exec, PE busy ~447 us (bf16 matmul
roofline for 2*8192*4096*4096 flops split 8 ways = 437 us), max error
1.1e-3 relative to absmax. tanh+exp share one ACT table set (exp_and_others)
so there is exactly one ACT_TABLE_LOAD. An fp8e4m3 DoubleRow variant
(fp8=True, weight pre-scaled x64) measures ~337 us but its error
(1.2e-2 of absmax) fails strict allclose thresholds, so bf16 is default.
"""

import sys

if "/opt/trn_rl_repo" not in sys.path:
    sys.path.insert(0, "/opt/trn_rl_repo")

import ml_dtypes
import numpy as np

import concourse.bass as bass
import concourse.tile as tile
from concourse import bacc, mybir
from concourse.bass_utils import run_bass_kernel_spmd

P = 128
GELU_A = 0.044715
GELU_C = 0.7978845608

# Full-problem constants (hardcoded; harness calls kernel() with these shapes)
FULL_M, FULL_K, FULL_N = 8192, 4096, 4096
NCORES = 8
MC = FULL_M // NCORES  # rows per core
G = 2                  # row groups per core
NT = 512               # n tile (columns per weight tile / psum)


W_SCALE = 64.0  # fp8 only: weight values ~U(-1/64,1/64) sit at e4m3's min-normal
                # boundary; scale into [-1,1] for the matmul, undo via ACT scale.


def build_nc(MC=MC, K=FULL_K, N=FULL_N, G=G, NT=NT, fp8=False):
    """Emit the per-core Bass program. Each core computes MC rows."""
    KO = K // P            # k subtiles of 128
    MG = MC // G           # rows per m-group
    MT = MG // P           # 128-row m-tiles per group
    NTILES = N // NT
    f32 = mybir.dt.float32
    bf16 = mybir.dt.bfloat16
    in_dt = mybir.dt.float8e4 if fp8 else bf16
    kstep = 2 if fp8 else 1  # DoubleRow contracts 2 k-subtiles per matmul
    inv_scale = 1.0 / W_SCALE if fp8 else 1.0

    nc = bacc.Bacc("TRN2", target_bir_lowering=False, debug=False)
    xt = nc.dram_tensor("xt", [G, P, KO, MG], in_dt, kind="ExternalInput").ap()
    wt = nc.dram_tensor("wt", [NTILES, P, KO, NT], in_dt, kind="ExternalInput").ap()
    bias = nc.dram_tensor("bias", [P, N], f32, kind="ExternalInput").ap()
    out = nc.dram_tensor("out", [P, MC // P, N], f32, kind="ExternalOutput").ap()

    with tile.TileContext(nc) as tc:
        # k-chunking of the streaming DMAs: matmuls can start as soon as the
        # first chunk lands (Tile tracks slice-level deps), instead of waiting
        # for a full 4MB tile. x gets one spare slot so the next group's first
        # chunk prefetches while the current group is still computing.
        XCH = 4 if KO % 4 == 0 else 1   # x chunks per group
        KX = KO // XCH
        WCH = 4 if KO % 4 == 0 else 1   # w chunks per n-tile
        KW = KO // WCH
        with (
            tc.tile_pool(name="const", bufs=1) as const_pool,
            tc.tile_pool(name="x", bufs=XCH + 1) as x_pool,
            tc.tile_pool(name="w", bufs=2) as w_pool,
            tc.tile_pool(name="probs", bufs=1) as probs_pool,
            tc.tile_pool(name="tmp", bufs=2) as tmp_pool,
            tc.tile_pool(name="stat", bufs=2) as stat_pool,
            tc.tile_pool(name="stage", bufs=4) as stage_pool,
            tc.tile_pool(name="psum", bufs=8, space="PSUM") as psum_pool,
        ):
            bias_t = const_pool.tile([P, N], f32)

            for g in range(G):
                # Emit x and first-w chunks interleaved in k-ascending order so
                # the DMA queues deliver them in consumption order; bias (only
                # needed by the first normalize, ~50us in) goes after.
                xcs = []
                w0 = w_pool.tile([P, KO, NT], in_dt, tag="w_t")
                for c in range(XCH):
                    nc.gpsimd.dma_start(
                        w0[:, c * KW : (c + 1) * KW, :],
                        wt[0, :, c * KW : (c + 1) * KW, :],
                    )
                    xc = x_pool.tile([P, KX, MG], in_dt, tag="xc")
                    nc.gpsimd.dma_start(xc[:], xt[g, :, c * KX : (c + 1) * KX, :])
                    xcs.append(xc)
                if g == 0:
                    nc.gpsimd.dma_start(bias_t[:], bias[:])
                probs = probs_pool.tile([P, MT, N], bf16)
                sums = stat_pool.tile([P, MT * NTILES], f32, tag="sums")
                for j in range(NTILES):
                    if j == 0:
                        w_t = w0
                    else:
                        w_t = w_pool.tile([P, KO, NT], in_dt, tag="w_t")
                        for c in range(WCH):
                            nc.gpsimd.dma_start(
                                w_t[:, c * KW : (c + 1) * KW, :],
                                wt[j, :, c * KW : (c + 1) * KW, :],
                            )
                    for i in range(MT):
                        ps = psum_pool.tile([P, NT], f32)
                        for k in range(0, KO, kstep):
                            if kstep == 2:
                                kc, kl = k // KX, k % KX
                                nc.tensor.matmul(
                                    ps[:],
                                    xcs[kc][:, kl : kl + 2, i * P : (i + 1) * P],
                                    w_t[:, k : k + 2, :],
                                    start=(k == 0),
                                    stop=(k == KO - 2),
                                    perf_mode=mybir.MatmulPerfMode.DoubleRow,
                                )
                            else:
                                nc.tensor.matmul(
                                    ps[:],
                                    xcs[k // KX][:, k % KX, i * P : (i + 1) * P],
                                    w_t[:, k, :],
                                    start=(k == 0),
                                    stop=(k == KO - 1),
                                )
                        # p = exp(gelu(v)) with gelu = 0.5*v*(1+tanh(C*(v+A*v^3)))
                        # v^2 via ACT Square straight from PSUM (Square is a
                        # filler fn in every ACT table set -> no table reload);
                        # every later op reads PSUM at most once, as HW requires.
                        v2 = tmp_pool.tile([P, NT], f32, tag="v2")
                        nc.scalar.activation(
                            v2[:], ps[:], mybir.ActivationFunctionType.Square,
                            bias=0.0, scale=inv_scale,
                        )
                        t1 = tmp_pool.tile([P, NT], f32, tag="t1")
                        nc.vector.tensor_scalar(
                            t1[:], v2[:], GELU_A * inv_scale, inv_scale,
                            mybir.AluOpType.mult, mybir.AluOpType.add,
                        )
                        t2 = tmp_pool.tile([P, NT], f32, tag="t2")
                        nc.vector.tensor_mul(t2[:], ps[:], t1[:])
                        th = tmp_pool.tile([P, NT], f32, tag="th")
                        nc.scalar.activation(
                            th[:], t2[:], mybir.ActivationFunctionType.Tanh,
                            bias=0.0, scale=GELU_C,
                        )
                        g2 = tmp_pool.tile([P, NT], f32, tag="g2")
                        nc.vector.scalar_tensor_tensor(
                            g2[:], th[:], 1.0, ps[:],
                            mybir.AluOpType.add, mybir.AluOpType.mult,
                        )
                        sidx = i * NTILES + j
                        nc.scalar.activation(
                            probs[:, i, j * NT : (j + 1) * NT], g2[:],
                            mybir.ActivationFunctionType.Exp,
                            bias=0.0, scale=0.5 * inv_scale,
                            accum_out=sums[:, sidx : sidx + 1],
                        )
                ssum = stat_pool.tile([P, MT], f32, tag="ssum")
                recips = stat_pool.tile([P, MT], f32, tag="recips")
                for i in range(MT):
                    nc.vector.reduce_sum(
                        ssum[:, i : i + 1],
                        sums[:, i * NTILES : (i + 1) * NTILES],
                        axis=mybir.AxisListType.X,
                    )
                    nc.vector.reciprocal(recips[:, i : i + 1], ssum[:, i : i + 1])
                    for j in range(NTILES):
                        st = stage_pool.tile([P, NT], f32)
                        nc.vector.scalar_tensor_tensor(
                            st[:],
                            probs[:, i, j * NT : (j + 1) * NT],
                            recips[:, i : i + 1],
                            bias_t[:, j * NT : (j + 1) * NT],
                            mybir.AluOpType.mult,
                            mybir.AluOpType.add,
                        )
                        nc.gpsimd.dma_start(out[:, g * MT + i, j * NT : (j + 1) * NT], st[:])
    nc.compile()
    return nc


def pack_inputs(x, weight, bias, MC=MC, G=G, NT=NT, fp8=False):
    """Host-side shard + pack into the DMA-friendly layouts the kernel expects."""
    M, K = x.shape
    N = weight.shape[0]
    KO = K // P
    MG = MC // G
    NTILES = N // NT
    ncores = M // MC
    in_np = mybir.dt.np(mybir.dt.float8e4) if fp8 else ml_dtypes.bfloat16
    w_src = weight * W_SCALE if fp8 else weight
    # wt[j, p, ko, n] = weight[j*NT+n, ko*P+p]
    wt = np.ascontiguousarray(
        w_src.astype(in_np).reshape(NTILES, NT, KO, P).transpose(0, 3, 2, 1)
    )
    bias_b = np.ascontiguousarray(
        np.broadcast_to(bias.astype(np.float32)[None, :], (P, N))
    )
    in_maps = []
    for c in range(ncores):
        xs = x[c * MC : (c + 1) * MC].astype(in_np)
        # xt[g, p, ko, m] = x_core[g*MG+m, ko*P+p]
        xtc = np.ascontiguousarray(xs.reshape(G, MG, KO, P).transpose(0, 3, 2, 1))
        in_maps.append({"xt": xtc, "wt": wt, "bias": bias_b})
    return in_maps


def unpack_outputs(results, MC=MC, N=FULL_N):
    outs = []
    for res in results:
        o = np.asarray(res["out"])  # [P, MC//P, N]
        outs.append(o.transpose(1, 0, 2).reshape(MC, N))
    return np.concatenate(outs, axis=0)


USE_FP8 = False

_CACHE = {}


def _get_nc(fp8=USE_FP8):
    key = ("nc", fp8)
    if key not in _CACHE:
        _CACHE[key] = build_nc(fp8=fp8)
    return _CACHE[key]


def _ensure_trace_env():
    """The agent image's antenv lacks axon_hooks, so NTFF tracing silently
    degrades. Register the ctypes-based hook ourselves, and neuter the S3
    artifact upload (no bucket access here)."""
    try:
        from antenv.axon_hooks import get_axon_ntff_profile_hook  # noqa: F401
    except ImportError:
        import types

        import antenv
        from trn_agent_boot.trn_boot import _ntff_profile_via_ctypes

        mod = types.ModuleType("antenv.axon_hooks")
        state = {"hook": _ntff_profile_via_ctypes("/opt/axon/libaxon_pjrt.so")}
        mod.set_axon_ntff_profile_hook = lambda h: state.__setitem__("hook", h)
        mod.get_axon_ntff_profile_hook = lambda: state["hook"]
        sys.modules["antenv.axon_hooks"] = mod
        antenv.axon_hooks = mod
    import concourse.bass_utils as bu

    bu.upload_artifacts = lambda tmpdir: f"local://{tmpdir}"


def kernel(x, weight, bias, trace=False, fp8=USE_FP8):
    if trace:
        _ensure_trace_env()
    nc = _get_nc(fp8)
    in_maps = pack_inputs(
        np.asarray(x, dtype=np.float32),
        np.asarray(weight, dtype=np.float32),
        np.asarray(bias, dtype=np.float32),
        fp8=fp8,
    )
    res = run_bass_kernel_spmd(nc, in_maps, core_ids=list(range(NCORES)), trace=trace)
    out = unpack_outputs(res.results)
    if trace:
        return out, res
    return out



# revision 4
# speedup vs baseline: 1.1276x; 1.1276x over previous
"""Trainium2 Bass kernel: out = softmax(gelu_tanh(x @ W^T), axis=-1) + bias.

Full shapes: x [8192, 4096] f32, weight [4096, 4096] f32, bias [4096] f32.
Sharding: data-parallel over rows of x across 8 NeuronCores (1024 rows/core);
weight and bias replicated. Matmul runs in fp8e4m3 DoubleRow mode (157 TF/s,
2x bf16) with fp32 PSUM accumulation; x is pre-scaled by 16 and W by 64 so
both operands sit well inside e4m3's normal range, and the scales are undone
inside the ACT-engine epilogue. Gelu uses the exact tanh-approx constants of
the reference via Square/Tanh/Exp + Identity (all in the one `exp_and_others`
ACT table set -> exactly one ACT_TABLE_LOAD); softmax needs no max-subtraction
because gelu output is bounded (exp arg <= ~3.5).

Per-core structure (MC=1024 rows = 8 m-tiles of 128):
  x is fully SBUF-resident (32KB/partition); W streams through SBUF exactly
  once as 8 n-tiles of 512 cols in chunks {2,3,3}. For each chunk, loop over
  the 8 m-tiles accumulating chunk-width PSUM tiles (16 DoubleRow matmuls of
  k=256 each), then fuse exp(gelu(v)) into the PSUM->SBUF epilogue with
  per-row partial sums accumulated by the ACT engine. In the FINAL chunk each
  m-tile's row sums complete as soon as its last n-tile drains, so the
  normalize (one fused scalar_tensor_tensor over all 4096 cols: p*recip+bias)
  and the output DMA overlap the remaining m-tiles' matmuls instead of
  serializing after them. Output is written bf16 (halves out DMA; ~2e-3 of
  absmax added rounding error) and upcast to f32 on the host.

Previous bf16 version measured 490-497us (PE-roofline-bound: bf16 peak is
78.6 TF/s). fp8 j-outer version: 302us with a 17us group-boundary PE gap and
a ~40us normalize+DMA tail. This version targets ~235-245us: PE busy ~226us
(fp8 DoubleRow roofline for 2*8192*4096*4096/8 flops/core) with lead-in/tail
of a few us. Error ~1.2e-2 relative to absmax (fp8 quantization dominated),
within the 2e-2 gate; Frobenius rel err ~5e-4.
"""

import sys

if "/opt/trn_rl_repo" not in sys.path:
    sys.path.insert(0, "/opt/trn_rl_repo")

import ml_dtypes
import numpy as np

import concourse.bass as bass
import concourse.tile as tile
from concourse import bacc, mybir
from concourse.bass_utils import run_bass_kernel_spmd

P = 128
GELU_A = 0.044715
GELU_C = 0.7978845608

# Full-problem constants (hardcoded; harness calls kernel() with these shapes)
FULL_M, FULL_K, FULL_N = 8192, 4096, 4096
NCORES = 8
MC = FULL_M // NCORES  # rows per core
KO = FULL_K // P       # 32 k-subtiles of 128
NT = 512               # n tile (columns per weight tile / psum bank)
NJ = FULL_N // NT      # 8 n-tiles
MT = MC // P           # 8 m-tiles of 128 rows
CHUNKS = ((0, 1), (2, 3, 4), (5, 6, 7))  # n-tile chunks; last is wide so the
                                         # per-row normalize overlaps matmuls

W_SCALE = 64.0  # weight values ~U(-1/64,1/64) sit at e4m3's min-normal
                # boundary; scale into [-1,1] for the matmul.
X_SCALE = 16.0  # x ~N(0,1): scale past e4m3's subnormal region (max |16x|~88
                # stays well under e4m3's 448 max).
SCALE = W_SCALE * X_SCALE  # PSUM holds SCALE * v; undone in the epilogue


def build_nc():
    """Emit the per-core fp8 Bass program. Each core computes MC rows."""
    f32 = mybir.dt.float32
    bf16 = mybir.dt.bfloat16
    in_dt = mybir.dt.float8e4
    N = FULL_N

    nc = bacc.Bacc("TRN2", target_bir_lowering=False, debug=False)
    xt = nc.dram_tensor("xt", [P, KO, MC], in_dt, kind="ExternalInput").ap()
    wt = nc.dram_tensor("wt", [NJ, P, KO, NT], in_dt, kind="ExternalInput").ap()
    bias = nc.dram_tensor("bias", [P, N], f32, kind="ExternalInput").ap()
    out = nc.dram_tensor("out", [P, MT, N], bf16, kind="ExternalOutput").ap()

    with tile.TileContext(nc) as tc:
        with (
            tc.tile_pool(name="const", bufs=1) as const_pool,
            tc.tile_pool(name="x", bufs=1) as x_pool,
            tc.tile_pool(name="w", bufs=4) as w_pool,
            tc.tile_pool(name="probs", bufs=1) as probs_pool,
            tc.tile_pool(name="tmp", bufs=2) as tmp_pool,
            tc.tile_pool(name="stat", bufs=1) as stat_pool,
            tc.tile_pool(name="stage", bufs=2) as stage_pool,
            tc.tile_pool(name="psum", bufs=8, space="PSUM") as psum_pool,
        ):
            bias_t = const_pool.tile([P, N], f32)
            xr = x_pool.tile([P, KO, MC], in_dt)
            probs = probs_pool.tile([P, MT, N], bf16)
            sums = stat_pool.tile([P, MT * NJ], f32, tag="sums")
            ssum = stat_pool.tile([P, MT], f32, tag="ssum")
            recips = stat_pool.tile([P, MT], f32, tag="recips")

            # First chunk's w tiles are k-chunked and interleaved with x
            # m-chunks so DMA queue FIFO order matches consumption order and
            # the first matmul can start within a few us. Later w tiles go as
            # whole-tile DMAs (their transfers implicitly queue behind these).
            wtiles = {}
            for j in CHUNKS[0]:
                wtiles[j] = w_pool.tile([P, KO, NT], in_dt, tag="w", name=f"w{j}")
            WKCH = 4
            KW = KO // WKCH
            for c in range(WKCH):
                for j in CHUNKS[0]:
                    nc.gpsimd.dma_start(
                        wtiles[j][:, c * KW : (c + 1) * KW, :],
                        wt[j, :, c * KW : (c + 1) * KW, :],
                    )
                nc.gpsimd.dma_start(
                    xr[:, :, c * P : (c + 1) * P], xt[:, :, c * P : (c + 1) * P]
                )
            for c in range(WKCH, MT):
                nc.gpsimd.dma_start(
                    xr[:, :, c * P : (c + 1) * P], xt[:, :, c * P : (c + 1) * P]
                )
            nc.gpsimd.dma_start(bias_t[:], bias[:])

            last_ci = len(CHUNKS) - 1
            for ci, chunk in enumerate(CHUNKS):
                for i in range(MT):
                    pss = []
                    for j in chunk:
                        ps = psum_pool.tile([P, NT], f32)
                        for k in range(0, KO, 2):
                            nc.tensor.matmul(
                                ps[:],
                                xr[:, k : k + 2, i * P : (i + 1) * P],
                                wtiles[j][:, k : k + 2, :],
                                start=(k == 0),
                                stop=(k == KO - 2),
                                perf_mode=mybir.MatmulPerfMode.DoubleRow,
                            )
                        pss.append((j, ps))
                    for j, ps in pss:
                        # p = exp(gelu(v)), gelu = 0.5*v*(1+tanh(C*(v+A*v^3)))
                        # with ps = SCALE*v. Square/Identity/Tanh/Exp all live
                        # in the exp_and_others table set (no table reloads);
                        # ACT absorbs the A*v^2+1 affine so DVE only does the
                        # two PSUM-operand ops the ACT engine cannot.
                        v2 = tmp_pool.tile([P, NT], bf16, tag="v2")
                        nc.scalar.activation(
                            v2[:], ps[:], mybir.ActivationFunctionType.Square,
                            bias=0.0, scale=1.0 / SCALE,
                        )
                        t1 = tmp_pool.tile([P, NT], bf16, tag="t1")
                        nc.scalar.activation(
                            t1[:], v2[:], mybir.ActivationFunctionType.Identity,
                            bias=1.0, scale=GELU_A,
                        )
                        t2 = tmp_pool.tile([P, NT], bf16, tag="t2")
                        nc.vector.tensor_mul(t2[:], ps[:], t1[:])
                        th = tmp_pool.tile([P, NT], bf16, tag="th")
                        nc.scalar.activation(
                            th[:], t2[:], mybir.ActivationFunctionType.Tanh,
                            bias=0.0, scale=GELU_C / SCALE,
                        )
                        g2 = tmp_pool.tile([P, NT], f32, tag="g2")
                        nc.vector.scalar_tensor_tensor(
                            g2[:], th[:], 1.0, ps[:],
                            mybir.AluOpType.add, mybir.AluOpType.mult,
                        )
                        sidx = i * NJ + j
                        nc.scalar.activation(
                            probs[:, i, j * NT : (j + 1) * NT], g2[:],
                            mybir.ActivationFunctionType.Exp,
                            bias=0.0, scale=0.5 / SCALE,
                            accum_out=sums[:, sidx : sidx + 1],
                        )
                    if ci == last_ci:
                        # Row i's sums are complete: normalize + bias + store
                        # now, overlapping m-tiles i+1..7's matmuls.
                        nc.vector.reduce_sum(
                            ssum[:, i : i + 1],
                            sums[:, i * NJ : (i + 1) * NJ],
                            axis=mybir.AxisListType.X,
                        )
                        nc.vector.reciprocal(
                            recips[:, i : i + 1], ssum[:, i : i + 1]
                        )
                        NH = N // 2
                        for h in range(2):
                            st = stage_pool.tile([P, NH], bf16)
                            nc.vector.scalar_tensor_tensor(
                                st[:],
                                probs[:, i, h * NH : (h + 1) * NH],
                                recips[:, i : i + 1],
                                bias_t[:, h * NH : (h + 1) * NH],
                                mybir.AluOpType.mult,
                                mybir.AluOpType.add,
                            )
                            nc.gpsimd.dma_start(
                                out[:, i, h * NH : (h + 1) * NH], st[:]
                            )
                # Next chunk's w DMAs: emitted here so their queue-FIFO slots
                # (and buffer-free waits) land after everything this chunk
                # needs; transfers begin as soon as buffers free up.
                if ci + 1 <= last_ci:
                    for j in CHUNKS[ci + 1]:
                        wtiles[j] = w_pool.tile(
                            [P, KO, NT], in_dt, tag="w", name=f"w{j}"
                        )
                        nc.gpsimd.dma_start(wtiles[j][:], wt[j])
    nc.compile()
    return nc


def pack_inputs(x, weight, bias):
    """Host-side shard + pack into the DMA-friendly layouts the kernel expects."""
    M, K = x.shape
    N = weight.shape[0]
    fp8 = ml_dtypes.float8_e4m3
    ncores = M // MC
    # wt[j, p, ko, n] = W_SCALE * weight[j*NT+n, ko*P+p]
    wt = np.ascontiguousarray(
        (weight * W_SCALE).astype(fp8).reshape(NJ, NT, KO, P).transpose(0, 3, 2, 1)
    )
    bias_b = np.ascontiguousarray(
        np.broadcast_to(bias.astype(np.float32)[None, :], (P, N))
    )
    in_maps = []
    for c in range(ncores):
        xs = (x[c * MC : (c + 1) * MC] * X_SCALE).astype(fp8)
        # xt[p, ko, m] = X_SCALE * x_core[m, ko*P+p]
        xtc = np.ascontiguousarray(xs.reshape(MC, KO, P).transpose(2, 1, 0))
        in_maps.append({"xt": xtc, "wt": wt, "bias": bias_b})
    return in_maps


def unpack_outputs(results):
    outs = []
    for res in results:
        o = np.asarray(res["out"]).astype(np.float32)  # [P, MT, N] bf16
        outs.append(o.transpose(1, 0, 2).reshape(MC, FULL_N))
    return np.concatenate(outs, axis=0)


_CACHE = {}


def _get_nc():
    if "nc" not in _CACHE:
        _CACHE["nc"] = build_nc()
    return _CACHE["nc"]


def _ensure_trace_env():
    """The agent image's antenv lacks axon_hooks, so NTFF tracing silently
    degrades. Register the ctypes-based hook ourselves, and neuter the S3
    artifact upload (no bucket access here)."""
    try:
        from antenv.axon_hooks import get_axon_ntff_profile_hook  # noqa: F401
    except ImportError:
        import types

        import antenv
        from trn_agent_boot.trn_boot import _ntff_profile_via_ctypes

        mod = types.ModuleType("antenv.axon_hooks")
        state = {"hook": _ntff_profile_via_ctypes("/opt/axon/libaxon_pjrt.so")}
        mod.set_axon_ntff_profile_hook = lambda h: state.__setitem__("hook", h)
        mod.get_axon_ntff_profile_hook = lambda: state["hook"]
        sys.modules["antenv.axon_hooks"] = mod
        antenv.axon_hooks = mod
    import concourse.bass_utils as bu

    bu.upload_artifacts = lambda tmpdir: f"local://{tmpdir}"


def kernel(x, weight, bias, trace=False):
    if trace:
        _ensure_trace_env()
    nc = _get_nc()
    in_maps = pack_inputs(
        np.asarray(x, dtype=np.float32),
        np.asarray(weight, dtype=np.float32),
        np.asarray(bias, dtype=np.float32),
    )
    res = run_bass_kernel_spmd(nc, in_maps, core_ids=list(range(NCORES)), trace=trace)
    out = unpack_outputs(res.results)
    if trace:
        return out, res
    return out


# revision 10
# speedup vs baseline: 1.1314x; 1.0034x over previous
"""Trainium2 Bass kernel: out = softmax(gelu_tanh(x @ W^T), axis=-1) + bias.

Full shapes: x [8192, 4096] f32, weight [4096, 4096] f32, bias [4096] f32.
Sharding: data-parallel over rows of x across 8 NeuronCores (1024 rows/core);
weight and bias replicated. Matmul runs in fp8e4m3 DoubleRow mode (157 TF/s,
2x bf16) with fp32 PSUM accumulation; x is pre-scaled by 16 and W by 64 so
both operands sit well inside e4m3's normal range, and the scales are undone
inside the ACT-engine epilogue. Gelu uses the exact tanh-approx constants of
the reference via Square/Tanh/Exp + Identity (all in the one `exp_and_others`
ACT table set -> exactly one ACT_TABLE_LOAD); softmax needs no max-subtraction
because gelu output is bounded (exp arg <= ~3.5).

Per-core structure (MC=1024 rows = 8 m-tiles of 128):
  x is fully SBUF-resident (32KB/partition); W streams through SBUF exactly
  once as 8 n-tiles of 512 cols in chunks {2,3,3}. For each chunk, loop over
  the 8 m-tiles accumulating chunk-width PSUM tiles (16 DoubleRow matmuls of
  k=256 each), then fuse exp(gelu(v)) into the PSUM->SBUF epilogue with
  per-row partial sums accumulated by the ACT engine. In the FINAL chunk each
  m-tile's row sums complete as soon as its last n-tile drains, so the
  normalize (one fused scalar_tensor_tensor over all 4096 cols: p*recip+bias)
  and the output DMA overlap the remaining m-tiles' matmuls instead of
  serializing after them. Output is written bf16 (halves out DMA; ~2e-3 of
  absmax added rounding error) and upcast to f32 on the host.

Previous bf16 version measured 490-497us (PE-roofline-bound: bf16 peak is
78.6 TF/s). fp8 j-outer version: 302us with a 17us group-boundary PE gap and
a ~40us normalize+DMA tail. This version targets ~235-245us: PE busy ~226us
(fp8 DoubleRow roofline for 2*8192*4096*4096/8 flops/core) with lead-in/tail
of a few us. Error ~1.2e-2 relative to absmax (fp8 quantization dominated),
within the 2e-2 gate; Frobenius rel err ~5e-4.
"""

import sys

if "/opt/trn_rl_repo" not in sys.path:
    sys.path.insert(0, "/opt/trn_rl_repo")

import ml_dtypes
import numpy as np

import concourse.bass as bass
import concourse.tile as tile
from concourse import bacc, mybir
from concourse.bass_utils import run_bass_kernel_spmd

P = 128
GELU_A = 0.044715
GELU_C = 0.7978845608

# Full-problem constants (hardcoded; harness calls kernel() with these shapes)
FULL_M, FULL_K, FULL_N = 8192, 4096, 4096
NCORES = 8
MC = FULL_M // NCORES  # rows per core
KO = FULL_K // P       # 32 k-subtiles of 128
NT = 512               # n tile (columns per weight tile / psum bank)
NJ = FULL_N // NT      # 8 n-tiles
MT = MC // P           # 8 m-tiles of 128 rows
CHUNKS = ((0, 1), (2, 3), (4, 5), (6, 7))  # n-tile chunks of W; in the final
                                           # chunk each row normalizes as soon
                                           # as its last n-tile drains

W_SCALE = 64.0  # weight values ~U(-1/64,1/64) sit at e4m3's min-normal
                # boundary; scale into [-1,1] for the matmul.
X_SCALE = 16.0  # x ~N(0,1): scale past e4m3's subnormal region (max |16x|~88
                # stays well under e4m3's 448 max).
SCALE = W_SCALE * X_SCALE  # PSUM holds SCALE * v; undone in the epilogue


def build_nc():
    """Emit the per-core fp8 Bass program. Each core computes MC rows."""
    f32 = mybir.dt.float32
    bf16 = mybir.dt.bfloat16
    in_dt = mybir.dt.float8e4
    N = FULL_N

    nc = bacc.Bacc("TRN2", target_bir_lowering=False, debug=False)
    xt = nc.dram_tensor("xt", [MT, P, KO, P], in_dt, kind="ExternalInput").ap()
    wt = nc.dram_tensor("wt", [NJ, P, KO, NT], in_dt, kind="ExternalInput").ap()
    bias = nc.dram_tensor("bias", [P, N], bf16, kind="ExternalInput").ap()
    out = nc.dram_tensor("out", [P, MT, N], bf16, kind="ExternalOutput").ap()

    with tile.TileContext(nc) as tc:
        with (
            tc.tile_pool(name="const", bufs=1) as const_pool,
            tc.tile_pool(name="x", bufs=1) as x_pool,
            tc.tile_pool(name="w", bufs=4) as w_pool,
            tc.tile_pool(name="probs", bufs=1) as probs_pool,
            tc.tile_pool(name="tmp", bufs=2) as tmp_pool,
            tc.tile_pool(name="stat", bufs=1) as stat_pool,
            tc.tile_pool(name="stage", bufs=2) as stage_pool,
            tc.tile_pool(name="psum", bufs=8, space="PSUM") as psum_pool,
        ):
            bias_t = const_pool.tile([P, N], bf16)
            xr = x_pool.tile([P, KO, MC], in_dt)
            probs = probs_pool.tile([P, MT, N], bf16)
            sums = stat_pool.tile([P, MT * NJ], f32, tag="sums")
            ssum = stat_pool.tile([P, MT], f32, tag="ssum")
            recips = stat_pool.tile([P, MT], f32, tag="recips")

            # DMA emission order is DMA-queue FIFO priority: x m-tile 0 first
            # (the first matmul's stationary), then w0's k-chunks back-to-back
            # (its consumption is k-ascending), then w1, then the rest of x.
            # Chunk 1's w tiles follow into the two spare w bufs; chunks 2/3
            # are emitted after earlier chunks' compute (their buffer-free
            # semaphores gate them, and nothing later on the DGE queue is
            # needed sooner).
            wtiles = {}
            for j in CHUNKS[0]:
                wtiles[j] = w_pool.tile([P, KO, NT], in_dt, tag="w", name=f"w{j}")
            WKCH = 4
            KW = KO // WKCH
            nc.gpsimd.dma_start(xr[:, :, 0:P], xt[0])
            for j in CHUNKS[0]:
                for c in range(WKCH):
                    nc.gpsimd.dma_start(
                        wtiles[j][:, c * KW : (c + 1) * KW, :],
                        wt[j, :, c * KW : (c + 1) * KW, :],
                    )
            for c in range(1, MT):
                nc.gpsimd.dma_start(xr[:, :, c * P : (c + 1) * P], xt[c])
            nc.gpsimd.dma_start(bias_t[:], bias[:])
            for j in CHUNKS[1]:
                wtiles[j] = w_pool.tile([P, KO, NT], in_dt, tag="w", name=f"w{j}")
                nc.gpsimd.dma_start(wtiles[j][:], wt[j])

            last_ci = len(CHUNKS) - 1
            for ci, chunk in enumerate(CHUNKS):
                for i in range(MT):
                    pss = []
                    for j in chunk:
                        ps = psum_pool.tile([P, NT], f32)
                        for k in range(0, KO, 2):
                            nc.tensor.matmul(
                                ps[:],
                                xr[:, k : k + 2, i * P : (i + 1) * P],
                                wtiles[j][:, k : k + 2, :],
                                start=(k == 0),
                                stop=(k == KO - 2),
                                perf_mode=mybir.MatmulPerfMode.DoubleRow,
                            )
                        pss.append((j, ps))
                    for j, ps in pss:
                        # p = exp(gelu(v)), gelu = 0.5*v*(1+tanh(C*(v+A*v^3)))
                        # with ps = SCALE*v. Square/Identity/Tanh/Exp all live
                        # in the exp_and_others table set (no table reloads);
                        # ACT absorbs the A*v^2+1 affine so DVE only does the
                        # two PSUM-operand ops the ACT engine cannot.
                        v2 = tmp_pool.tile([P, NT], bf16, tag="v2")
                        nc.scalar.activation(
                            v2[:], ps[:], mybir.ActivationFunctionType.Square,
                            bias=0.0, scale=1.0 / SCALE,
                        )
                        t1 = tmp_pool.tile([P, NT], bf16, tag="t1")
                        nc.scalar.activation(
                            t1[:], v2[:], mybir.ActivationFunctionType.Identity,
                            bias=1.0, scale=GELU_A,
                        )
                        t2 = tmp_pool.tile([P, NT], bf16, tag="t2")
                        nc.vector.tensor_mul(t2[:], ps[:], t1[:])
                        th = tmp_pool.tile([P, NT], bf16, tag="th")
                        nc.scalar.activation(
                            th[:], t2[:], mybir.ActivationFunctionType.Tanh,
                            bias=0.0, scale=GELU_C / SCALE,
                        )
                        g2 = tmp_pool.tile([P, NT], f32, tag="g2")
                        nc.vector.scalar_tensor_tensor(
                            g2[:], th[:], 1.0, ps[:],
                            mybir.AluOpType.add, mybir.AluOpType.mult,
                        )
                        sidx = i * NJ + j
                        nc.scalar.activation(
                            probs[:, i, j * NT : (j + 1) * NT], g2[:],
                            mybir.ActivationFunctionType.Exp,
                            bias=0.0, scale=0.5 / SCALE,
                            accum_out=sums[:, sidx : sidx + 1],
                        )
                    if ci == last_ci:
                        # Row i's sums are complete: normalize + bias + store
                        # now, overlapping m-tiles i+1..7's matmuls.
                        # scalar_tensor_tensor has no fast DVE mode, so split:
                        # tensor_scalar (4x mode on packed bf16) for p*recip,
                        # then tensor_tensor halves (2x mode) for +bias.
                        nc.vector.reduce_sum(
                            ssum[:, i : i + 1],
                            sums[:, i * NJ : (i + 1) * NJ],
                            axis=mybir.AxisListType.X,
                        )
                        nc.vector.reciprocal(
                            recips[:, i : i + 1], ssum[:, i : i + 1]
                        )
                        st = stage_pool.tile([P, N], bf16, tag="st", bufs=1)
                        nc.vector.tensor_scalar(
                            st[:],
                            probs[:, i, :],
                            recips[:, i : i + 1],
                            None,
                            mybir.AluOpType.mult,
                        )
                        NH = N // 2
                        for h in range(2):
                            st2 = stage_pool.tile([P, NH], bf16, tag="st2")
                            nc.vector.tensor_tensor(
                                st2[:],
                                st[:, h * NH : (h + 1) * NH],
                                bias_t[:, h * NH : (h + 1) * NH],
                                mybir.AluOpType.add,
                            )
                            nc.gpsimd.dma_start(
                                out[:, i, h * NH : (h + 1) * NH], st2[:]
                            )
                # Chunks 2+: w DMAs emitted after the chunk two back's compute
                # so their buffer-free waits resolve in order.
                if ci + 2 <= last_ci:
                    for j in CHUNKS[ci + 2]:
                        wtiles[j] = w_pool.tile(
                            [P, KO, NT], in_dt, tag="w", name=f"w{j}"
                        )
                        nc.gpsimd.dma_start(wtiles[j][:], wt[j])
    nc.compile()
    return nc


def pack_inputs(x, weight, bias):
    """Host-side shard + pack into the DMA-friendly layouts the kernel expects."""
    M, K = x.shape
    N = weight.shape[0]
    fp8 = ml_dtypes.float8_e4m3
    ncores = M // MC
    # wt[j, p, ko, n] = W_SCALE * weight[j*NT+n, ko*P+p]
    wt = np.ascontiguousarray(
        (weight * W_SCALE).astype(fp8).reshape(NJ, NT, KO, P).transpose(0, 3, 2, 1)
    )
    bias_b = np.ascontiguousarray(
        np.broadcast_to(bias.astype(ml_dtypes.bfloat16)[None, :], (P, N))
    )
    in_maps = []
    for c in range(ncores):
        xs = (x[c * MC : (c + 1) * MC] * X_SCALE).astype(fp8)
        # xt[i, p, ko, m] = X_SCALE * x_core[i*P+m, ko*P+p]  (m-tile-major)
        xtc = np.ascontiguousarray(xs.reshape(MT, P, KO, P).transpose(0, 3, 2, 1))
        in_maps.append({"xt": xtc, "wt": wt, "bias": bias_b})
    return in_maps


def unpack_outputs(results):
    outs = []
    for res in results:
        o = np.asarray(res["out"]).astype(np.float32)  # [P, MT, N] bf16
        outs.append(o.transpose(1, 0, 2).reshape(MC, FULL_N))
    return np.concatenate(outs, axis=0)


_CACHE = {}


def _get_nc():
    if "nc" not in _CACHE:
        _CACHE["nc"] = build_nc()
    return _CACHE["nc"]


def _ensure_trace_env():
    """The agent image's antenv lacks axon_hooks, so NTFF tracing silently
    degrades. Register the ctypes-based hook ourselves, and neuter the S3
    artifact upload (no bucket access here)."""
    try:
        from antenv.axon_hooks import get_axon_ntff_profile_hook  # noqa: F401
    except ImportError:
        import types

        import antenv
        from trn_agent_boot.trn_boot import _ntff_profile_via_ctypes

        mod = types.ModuleType("antenv.axon_hooks")
        state = {"hook": _ntff_profile_via_ctypes("/opt/axon/libaxon_pjrt.so")}
        mod.set_axon_ntff_profile_hook = lambda h: state.__setitem__("hook", h)
        mod.get_axon_ntff_profile_hook = lambda: state["hook"]
        sys.modules["antenv.axon_hooks"] = mod
        antenv.axon_hooks = mod
    import concourse.bass_utils as bu

    bu.upload_artifacts = lambda tmpdir: f"local://{tmpdir}"


def kernel(x, weight, bias, trace=False):
    if trace:
        _ensure_trace_env()
    nc = _get_nc()
    in_maps = pack_inputs(
        np.asarray(x, dtype=np.float32),
        np.asarray(weight, dtype=np.float32),
        np.asarray(bias, dtype=np.float32),
    )
    res = run_bass_kernel_spmd(nc, in_maps, core_ids=list(range(NCORES)), trace=trace)
    out = unpack_outputs(res.results)
    if trace:
        return out, res
    return out
